# revision 3
# baseline (speedup 1.0000x reference)
"""Trainium2 Bass kernel for nn_Net_3659312136203 — v2.

Data-parallel over batch (8192 -> 8 cores x 1024). Per core, 96-step scan
with two independent 512-row groups software-pipelined so the PE never
starves (HAM stays at K=8/8).

Per step, per group g (batch blocks j=0..3, 128 rows each):
  - state math batch-major on [128, 4] tiles (DVE/ACT/GPSIMD)
  - aout/ns written interleaved into asn [128, 8] f32, cast to bf16
  - fold-in: ONE PE transpose [128,8] -> [8,128] psum (bf16) + evac
  - h1 = W1f @ feat (N=512) + W1as @ asT[2j:2j+2] (4 MMs N=128), accumulated
  - h2 = W2 @ h1s: 4 MMs N=512 (f32 psum)
  - w3 batch-major: lhsT = h2s[:, 128j:...] slices, rhs = w3 cols ->
    psum amlT [128, 4] directly batch-major (16 LDW+MM pairs, N=1)
  - a_ml = relu(psum + b3) fused in the ACT evac
  - dev@q_col / dev@g_col matvecs replaced by geometric recurrence
    s_t = ad_t + 0.25 s_{t-1} (cum_d = 2 ad + 0.375 s_prev; cum_dg = cg[t] s_t)
  - last step: only a_out is live; state/bgt/cum updates skipped
"""
import sys
import os

sys.path.insert(0, "/opt/trn_rl_repo")

import numpy as np
import ml_dtypes

D1, D2, D3 = 0.1, 1.0, 2.0
POWER = 10.0
STATE_CAP = 15.0
NCORES = 8

_CACHE = {}


def _scalars(H, lam, bud):
    t = np.arange(H)
    S = (1.0 - 0.25 ** (H - 1.0 - t)) / 0.75
    off = D1 / 8.0 * 10.0 + D2 / 4.0  # 0.375
    diag = 2.0 * D1 * 5.0 + D2  # 2.0
    gamma = (diag + off * S).astype(np.float32)
    cg = (off * S).astype(np.float32)
    inv_g = (1.0 / gamma.astype(np.float64)).astype(np.float32)
    lam32 = np.float32(lam)
    bud32 = np.float32(bud)
    per_step = np.float32(lam32 * np.float32(D3) + bud32 / np.float32(H))
    onelam = np.float32(np.float32(1.0) + lam32)
    econ = (lam32 * np.float32(D3)
            + (bud32 / np.float32(H)) * (t + 2.0).astype(np.float32)).astype(np.float32)
    return gamma, inv_g, cg, per_step, onelam, econ


def _build_program(H, lam, bud, b3v, mmdt_name):
    import concourse.tile as tile
    from concourse import bacc, mybir
    from contextlib import ExitStack

    f32 = mybir.dt.float32
    bf16 = mybir.dt.bfloat16
    mmdt = {"bf16": bf16, "f32": f32}[mmdt_name]
    Alu = mybir.AluOpType
    Act = mybir.ActivationFunctionType

    gamma, inv_g, cg, per_step, onelam, econ = _scalars(H, lam, bud)
    gamma = [float(x) for x in gamma]
    inv_g = [float(x) for x in inv_g]
    cg = [float(x) for x in cg]
    econ = [float(x) for x in econ]
    per_step = float(per_step)
    onelam = float(onelam)

    nc = bacc.Bacc("TRN2", target_bir_lowering=False, debug=False,
                   enable_asserts=False)

    featT_d = nc.dram_tensor("featT", [H, 4, 1024], mmdt, kind="ExternalInput")
    nzd_d = nc.dram_tensor("nzd", [H, 128, 24], f32, kind="ExternalInput")
    as0_d = nc.dram_tensor("as0", [128, 16], f32, kind="ExternalInput")
    w1f_d = nc.dram_tensor("w1f", [4, 256], mmdt, kind="ExternalInput")
    w1as_d = nc.dram_tensor("w1as", [2, 256], mmdt, kind="ExternalInput")
    w1x_d = nc.dram_tensor("w1x", [6, 256], mmdt, kind="ExternalInput")
    w2a_d = nc.dram_tensor("w2a", [128, 256], mmdt, kind="ExternalInput")
    w2b_d = nc.dram_tensor("w2b", [128, 256], mmdt, kind="ExternalInput")
    w3_d = nc.dram_tensor("w3c", [128, 2], mmdt, kind="ExternalInput")
    b12_d = nc.dram_tensor("b12", [128, 4], f32, kind="ExternalInput")
    id_d = nc.dram_tensor("ident", [128, 128], mmdt, kind="ExternalInput")
    out_d = nc.dram_tensor("outb", [H, 128, 8], f32, kind="ExternalOutput")

    def mm(out, lhsT, rhs, **kw):
        nc.tensor.matmul(out, lhsT, rhs, **kw)

    with ExitStack() as ctx:
        tc = ctx.enter_context(tile.TileContext(nc))
        P = lambda name, bufs, **kw: ctx.enter_context(
            tc.tile_pool(name=name, bufs=bufs, **kw))

        consts = P("consts", 1)
        ftp = P("ftp", 3)
        nzp = P("nzp", 3)
        asnp = P("asnp", 3)     # asn f32 [128, 16] merged (a,s interleaved)
        asnbp = P("asnbp", 4)   # per-group bf16 [128, 8]
        x2p = P("x2p", 4)       # per-group x2 [2, 512]
        bgp = P("bgp", 3)
        ccp_ = P("ccp", 3)
        sap = P("sap", 3)
        h1p_ = P("h1sb", 3)
        h2p_ = P("h2sb", 3)
        tmp = P("tmp", 6)
        # PSUM banks: ph1 2 + ph2 2 + pml 2 (per-group) + pT 2 (per-group)
        ph1 = P("ph1", 2, space="PSUM")
        ph2 = P("ph2", 2, space="PSUM")
        pml = P("pml", 1, space="PSUM")
        pT = P("pT", 1, space="PSUM")

        w1f = consts.tile([4, 256], mmdt)
        nc.sync.dma_start(w1f[:], w1f_d.ap())
        w1as = consts.tile([2, 256], mmdt)
        nc.sync.dma_start(w1as[:], w1as_d.ap())
        w2a = consts.tile([128, 256], mmdt)
        nc.sync.dma_start(w2a[:], w2a_d.ap())
        w2b = consts.tile([128, 256], mmdt)
        nc.sync.dma_start(w2b[:], w2b_d.ap())
        w3 = consts.tile([128, 2], mmdt)
        nc.sync.dma_start(w3[:], w3_d.ap())
        b12 = consts.tile([128, 4], f32)
        nc.sync.dma_start(b12[:], b12_d.ap())
        ident = consts.tile([128, 128], mmdt)
        nc.sync.dma_start(ident[:], id_d.ap())
        psb = consts.tile([128, 1], f32)
        nc.vector.memset(psb[:], per_step)
        w1x = consts.tile([6, 256], mmdt)
        nc.sync.dma_start(w1x[:], w1x_d.ap())

        as0sb = consts.tile([128, 16], f32)
        nc.sync.dma_start(as0sb[:], as0_d.ap())
        asn_prev = [None]
        a0i = asnp.tile([128, 16], f32, tag="asn", name="asn_init")
        nc.vector.tensor_copy(a0i[:], as0sb[:])
        asn_prev[0] = a0i
        bgt = [bgp.tile([128, 8], f32, tag="bg", name="bg_init")]
        cumc = [ccp_.tile([128, 8], f32, tag="cc", name="cc_init")]
        sacc = [sap.tile([128, 8], f32, tag="sa", name="sa_init")]
        nc.vector.memset(bgt[0][:], per_step)
        nc.gpsimd.memset(cumc[0][:], 0.0)
        nc.gpsimd.memset(sacc[0][:], 0.0)

        v, sc, gp, te = nc.vector, nc.scalar, nc.gpsimd, nc.tensor

        h1s_cur = [None, None]
        h2s_cur = [None, None]
        h1p_cur = [None, None]
        pml_cur = [None, None]
        x2_cur = [None, None]
        ft_cur = [None]
        nz_cur = [None]
        prep_ctx = [None, None]
        apm_cur = [None]

        ab_cur = [None, None]

        def fold_cast(g, t):
            ab = asnbp.tile([128, 8], bf16, tag=f"asnb{g}", name=f"asnb{g}_{t}")
            v.tensor_copy(ab[:], asn_prev[0][:, 8 * g:8 * g + 8])
            ab_cur[g] = ab

        def fold_T(g, t):
            ab = ab_cur[g]
            pt_ = pT.tile([2, 512], bf16, tag=f"pT{g}", name=f"pT{g}_{t}")
            for jj in range(4):
                te.transpose(pt_[:, 128 * jj:128 * (jj + 1)],
                             ab[:, 2 * jj:2 * jj + 2], ident[:])
            x6 = x2p.tile([6, 512], bf16, tag=f"x2{g}", name=f"x2{g}_{t}")
            nc.sync.dma_start(x6[2:6, :], featT_d.ap()[t, :, 512 * g:512 * (g + 1)])
            v.tensor_copy(x6[0:2, :], pt_[:])
            x2_cur[g] = x6

        def h1_as(g, t):
            x6 = x2_cur[g]
            h1s = []
            for mt in range(2):
                hp = ph1.tile([128, 512], f32, tag="h1", name=f"h1_{g}_{t}_{mt}")
                mm(hp[:], w1x[:, 128 * mt:128 * (mt + 1)], x6[:],
                   start=True, stop=True)
                hs = h1p_.tile([128, 512], mmdt, tag=f"h1s_{g}",
                               name=f"h1s_{g}_{t}_{mt}")
                if mt == 0:
                    sc.activation(hs[:], hp[:], Act.Relu, bias=b12[:, 0:1])
                else:
                    v.tensor_scalar(hs[:], hp[:], b12[:, 1:2], 0.0,
                                    op0=Alu.add, op1=Alu.max)
                h1s.append(hs)
            h1s_cur[g] = h1s

        h2p_cur = [None, None]

        def mlp_h2(g, t, ev1_inline=True):
            h1s = h1s_cur[g]
            h2s = []
            hps = []
            for mt in range(2):
                hp = ph2.tile([128, 512], f32, tag="h2", name=f"h2_{g}_{t}_{mt}")
                mm(hp[:], w2a[:, 128 * mt:128 * (mt + 1)], h1s[0][:],
                   start=True, stop=False)
                mm(hp[:], w2b[:, 128 * mt:128 * (mt + 1)], h1s[1][:],
                   start=False, stop=True)
                hps.append(hp)
                hs = h2p_.tile([128, 512], mmdt, tag=f"h2s_{g}",
                               name=f"h2s_{g}_{t}_{mt}")
                h2s.append(hs)
            sc.activation(h2s[0][:], hps[0][:], Act.Relu, bias=b12[:, 2:3])
            if ev1_inline:
                v.tensor_scalar(h2s[1][:], hps[1][:], b12[:, 3:4], 0.0,
                                op0=Alu.add, op1=Alu.max)
            h2p_cur[g] = hps
            h2s_cur[g] = h2s

        def h2_ev1(g, t):
            sc.activation(h2s_cur[g][1][:], h2p_cur[g][1][:], Act.Relu,
                          bias=b12[:, 3:4])

        def w3p0(g, t):
            h2s = h2s_cur[g]
            pm = pml.tile([128, 4], f32, tag=f"ml{g}", name=f"ml{g}_{t}")
            for jj in range(4):
                mm(pm[:, jj:jj + 1], h2s[0][:, 128 * jj:128 * (jj + 1)],
                   w3[:, 0:1], start=(jj == 0), stop=False)
            pml_cur[g] = pm

        def w3p1(g, t):
            h2s = h2s_cur[g]
            pm = pml_cur[g]
            for jj in range(4):
                mm(pm[:, jj:jj + 1], h2s[1][:, 128 * jj:128 * (jj + 1)],
                   w3[:, 1:2], start=False, stop=(jj == 3))

        def prep(g, t):
            """Pre-a_ml per-group quantities, mostly on GPSIMD (idle engine).
            ap goes into merged apm [128,8] halves for the merged tail."""
            nz = nz_cur[0]
            T4 = lambda tag: tmp.tile([128, 4], f32, tag=f"{tag}{g}",
                                      name=f"{tag}{g}_{t}")
            dem = nz[:, 4 * g:4 * g + 4]
            m2 = nz[:, 8 + 4 * g:12 + 4 * g]
            st_prev = asn_prev[0][:, 8 * g + 1:8 * g + 8:2]
            sd = T4("sd")
            gp.tensor_tensor(sd[:], st_prev, dem, op=Alu.add)
            ap0 = T4("ap0")
            gp.tensor_scalar(ap0[:], sd[:], 1.25, 0.0, op0=Alu.mult, op1=Alu.max)
            apm = apm_cur[0]
            ap = apm[:, 4 * g:4 * g + 4]
            gp.tensor_scalar(ap, ap0[:], POWER, None, op0=Alu.min)
            c = T4("c")
            gp.tensor_scalar_mul(c[:], bgt[0][:, 4 * g:4 * g + 4], inv_g[t])
            lo = T4("lo")
            gp.tensor_tensor(lo[:], ap, c[:], op=Alu.subtract)
            lo0 = T4("lo0")
            gp.tensor_scalar(lo0[:], lo[:], 0.0, None, op0=Alu.max)
            hi = T4("hi")
            gp.tensor_tensor(hi[:], ap, c[:], op=Alu.add)
            z1 = T4("z1")
            gp.tensor_tensor(z1[:], st_prev, m2, op=Alu.mult)
            z2 = T4("z2")
            gp.tensor_tensor(z2[:], z1[:], dem, op=Alu.add)
            prep_ctx[g] = (lo0, hi, z2)

        def crit(g, t, asn):
            """Critical chain: t1 = max(pml+b3, lo0); aout = min(t1, hi);
            ns = clip(z2 - mn*aout, 0, 15)."""
            last = (t == H - 1)
            nz = nz_cur[0]
            lo0, hi, z2 = prep_ctx[g]
            T4 = lambda tag: tmp.tile([128, 4], f32, tag=f"{tag}{g}",
                                      name=f"{tag}{g}_{t}")
            mn = nz[:, 16 + 4 * g:20 + 4 * g]
            t1 = T4("t1")
            v.scalar_tensor_tensor(t1[:], pml_cur[g][:], b3v, lo0[:],
                                   op0=Alu.add, op1=Alu.max)
            aout = asn[:, 8 * g:8 * g + 8:2]
            v.tensor_tensor(aout, t1[:], hi[:], op=Alu.min)
            if not last:
                z3 = T4("z3")
                v.tensor_tensor(z3[:], mn, aout, op=Alu.mult)
                z4 = T4("z4")
                v.tensor_tensor(z4[:], z2[:], z3[:], op=Alu.subtract)
                ns = asn[:, 8 * g + 1:8 * g + 8:2]
                v.tensor_scalar(ns, z4[:], 0.0, STATE_CAP, op0=Alu.max, op1=Alu.min)

        def state_tail(t, asn):
            if t == H - 1:
                return
            apm = apm_cur[0]
            T8 = lambda tag: tmp.tile([128, 8], f32, tag=tag, name=f"{tag}_{t}")
            aout = asn[:, 0:16:2]
            ns = asn[:, 1:16:2]
            dd = T8("dd")
            v.tensor_tensor(dd[:], aout, apm[:], op=Alu.subtract)
            ad = T8("ad")
            sc.activation(ad[:], dd[:], Act.Abs)
            sq = T8("sq")
            sc.activation(sq[:], ns, Act.Square)
            ccx = T8("ccx")
            sc.activation(ccx[:], sq[:], Act.Copy, bias=float(D3), scale=float(D1))
            c_cost = T8("cco")
            gp.tensor_tensor(c_cost[:], ccx[:], ns, op=Alu.add)
            u1 = T8("u1")
            v.scalar_tensor_tensor(u1[:], ad[:], -2.0, c_cost[:],
                                   op0=Alu.mult, op1=Alu.add)
            u2 = T8("u2")
            v.scalar_tensor_tensor(u2[:], sacc[0][:], -0.375, u1[:],
                                   op0=Alu.mult, op1=Alu.add)
            sn = sap.tile([128, 8], f32, tag="sa", name=f"sa_{t}")
            v.scalar_tensor_tensor(sn[:], sacc[0][:], 0.25, ad[:],
                                   op0=Alu.mult, op1=Alu.add)
            cp1 = T8("cp1")
            v.tensor_scalar(cp1[:], u2[:], 2.0, onelam, op0=Alu.max, op1=Alu.mult)
            q2 = T8("q2")
            gp.tensor_tensor(q2[:], cumc[0][:], cp1[:], op=Alu.add)
            ccn = ccp_.tile([128, 8], f32, tag="cc", name=f"cc_{t}")
            gp.tensor_tensor(ccn[:], q2[:], c_cost[:], op=Alu.subtract)
            v1 = T8("v1")
            v.scalar_tensor_tensor(v1[:], ad[:], -gamma[t], bgt[0][:],
                                   op0=Alu.mult, op1=Alu.add)
            e1 = T8("e1")
            sc.activation(e1[:], v1[:], Act.Relu, bias=psb[:, 0:1])
            v2 = T8("v2")
            v.scalar_tensor_tensor(v2[:], sn[:], -cg[t], ccn[:],
                                   op0=Alu.mult, op1=Alu.add)
            bn = bgp.tile([128, 8], f32, tag="bg", name=f"bg_{t}")
            v.scalar_tensor_tensor(bn[:], v2[:], econ[t], e1[:],
                                   op0=Alu.add, op1=Alu.max)
            bgt[0] = bn
            cumc[0] = ccn
            sacc[0] = sn

        NHEAT = int(os.environ.get("KHEAT", "0"))
        NWARM = int(os.environ.get("KWARM", "0"))

        def heat(tag, t, n):
            if n <= 0:
                return
            htile = pT.tile([2, 256], f32, tag=tag, name=f"heat_{tag}_{t}")
            for i in range(n):
                mm(htile[:], ident[:, 0:2], w2a[:, 0:256],
                   start=True, stop=True)

        def load_inputs(t):
            nz = nzp.tile([128, 24], f32, tag="nz", name=f"nz_{t}")
            nc.sync.dma_start(nz[:], nzd_d.ap()[t])
            return None, nz

        ft0, nz0 = load_inputs(0)
        if NWARM:
            for i in range(NWARM):
                wp = ph1.tile([128, 256], f32, tag="h1", name=f"warm_{i}")
                mm(wp[:], w2a[:, 0:128], w2b[:, 0:256], start=True, stop=True)
        fold_cast(0, 0)
        fold_cast(1, 0)
        fold_T(0, 0)
        nz_t = nz0

        for t in range(H):
            nz_cur[0] = nz_t
            apm_cur[0] = tmp.tile([128, 8], f32, tag="apm", name=f"apm_{t}")
            asn = asnp.tile([128, 16], f32, tag="asn", name=f"asn_{t}")
            h1_as(0, t)
            fold_T(1, t)
            prep(0, t)
            h1_as(1, t)
            prep(1, t)
            mlp_h2(0, t, ev1_inline=True)
            mlp_h2(1, t, ev1_inline=False)
            w3p0(0, t)
            if t + 1 < H:
                ftn, nzn = load_inputs(t + 1)
                ft_cur[0] = ftn
            w3p0(1, t)
            w3p1(0, t)
            crit(0, t, asn)
            asn_prev[0] = asn
            if t + 1 < H:
                fold_cast(0, t + 1)
            h2_ev1(1, t)
            heat("pT0", t, NHEAT)
            if t + 1 < H:
                fold_T(0, t + 1)
            w3p1(1, t)
            crit(1, t, asn)
            if t + 1 < H:
                fold_cast(1, t + 1)
            state_tail(t, asn)
            nc.sync.dma_start(out_d.ap()[t], asn[:, 0:16:2])
            if t + 1 < H:
                nz_t = nzn

    nc.compile()
    return nc


def _prep_core(pc, tn, dn, ap_, sp_, mmdt_name):
    """Per-core input arrays. pc: [1024, 96, 4]; tn/dn: [1024, 96]; ap_/sp_: [1024]."""
    H = pc.shape[1]
    mmnp = ml_dtypes.bfloat16 if mmdt_name == "bf16" else np.float32
    featT = np.ascontiguousarray(pc.transpose(1, 2, 0)).astype(mmnp)
    def bm(a):  # [1024, H] -> [H, 128, 8]
        return np.ascontiguousarray(
            a.reshape(2, 4, 128, H).transpose(3, 2, 0, 1).reshape(H, 128, 8))
    dem = bm(pc[:, :, 0])
    m2 = bm(1.0 - dn)
    mn = bm(0.8 + tn)
    nzd = np.ascontiguousarray(
        np.concatenate([dem, m2, mn], axis=2)).astype(np.float32)
    a = ap_.reshape(2, 4, 128)
    s = sp_.reshape(2, 4, 128)
    as0 = np.ascontiguousarray(
        np.stack([a, s], -1).transpose(2, 0, 1, 3).reshape(128, 16)).astype(np.float32)
    return featT, nzd, as0


def _prepare(policy_in_c, trans_noise, demand_noise, action_pre, state_pre,
             Lambda, Budget, W1, b1, W2, b2, W3, b3):
    mmdt_name = os.environ.get("KBASS_DT", "bf16")
    pc = np.asarray(policy_in_c, np.float32)
    tn = np.asarray(trans_noise, np.float32)[..., 0]
    dn = np.asarray(demand_noise, np.float32)[..., 0]
    ap_ = np.asarray(action_pre, np.float32)[:, 0]
    sp_ = np.asarray(state_pre, np.float32)[:, 0]
    lam = float(np.asarray(Lambda, np.float32)[0])
    bud = float(np.asarray(Budget, np.float32)[0])
    W1 = np.asarray(W1, np.float32)
    b1 = np.asarray(b1, np.float32)
    W2 = np.asarray(W2, np.float32)
    b2 = np.asarray(b2, np.float32)
    W3 = np.asarray(W3, np.float32)
    b3v = float(np.asarray(b3, np.float32)[0])
    B, H, C = pc.shape
    Bc = B // NCORES

    key = (H, lam, bud, b3v, mmdt_name, "v2")
    if key not in _CACHE:
        _CACHE[key] = _build_program(H, lam, bud, b3v, mmdt_name)
    nc = _CACHE[key]

    mmnp = ml_dtypes.bfloat16 if mmdt_name == "bf16" else np.float32
    # x rows: [action, state, feat0..3]; reference x = [feat, action, state]
    # so W1 cols 0:4 = feat, col 4 = action, col 5 = state.
    w1f = np.ascontiguousarray(W1[:, 0:4].T).astype(mmnp)     # [4, 256]
    w1as = np.ascontiguousarray(W1[:, 4:6].T).astype(mmnp)    # [2, 256]
    w1x = np.ascontiguousarray(W1[:, [4, 5, 0, 1, 2, 3]].T).astype(mmnp)  # [6, 256]
    w2t = np.ascontiguousarray(W2.T).astype(mmnp)
    w2a_np, w2b_np = w2t[0:128], w2t[128:256]
    w3c = np.ascontiguousarray(np.stack([W3[0, 0:128], W3[0, 128:256]], 1)).astype(mmnp)
    b12 = np.ascontiguousarray(
        np.stack([b1[0:128], b1[128:256], b2[0:128], b2[128:256]], 1)).astype(np.float32)
    ident = np.eye(128, dtype=np.float32).astype(mmnp)

    in_maps = []
    for cid in range(NCORES):
        sl = slice(cid * Bc, (cid + 1) * Bc)
        featT, nzd, as0 = _prep_core(pc[sl], tn[sl], dn[sl], ap_[sl], sp_[sl],
                                     mmdt_name)
        in_maps.append({
            "featT": featT, "nzd": nzd, "as0": as0,
            "w1f": w1f, "w1as": w1as, "w1x": w1x, "w2a": w2a_np, "w2b": w2b_np,
            "w3c": w3c, "b12": b12, "ident": ident,
        })

    return nc, in_maps, B, H, Bc


def _assemble(res, B, H, Bc):
    out = np.empty((B, H), np.float32)
    for cid in range(NCORES):
        ob = res.results[cid]["outb"]  # [H, 128, 8]
        oc = ob.reshape(H, 128, 2, 4).transpose(2, 3, 1, 0).reshape(Bc, H)
        out[cid * Bc:(cid + 1) * Bc] = oc
    return out


def kernel(**inputs):
    from concourse.bass_utils import run_bass_kernel_spmd
    nc, in_maps, B, H, Bc = _prepare(**inputs)
    res = run_bass_kernel_spmd(nc, in_maps, core_ids=list(range(NCORES)))
    return _assemble(res, B, H, Bc)



# revision 11
# speedup vs baseline: 1.1572x; 1.1572x over previous
"""Trainium2 Bass kernel for nn_Net_3659312136203 — v2.

Data-parallel over batch (8192 -> 8 cores x 1024). Per core, 96-step scan
with two independent 512-row groups software-pipelined so the PE never
starves (HAM stays at K=8/8).

Per step, per group g (batch blocks j=0..3, 128 rows each):
  - state math batch-major on [128, 4] tiles (DVE/ACT/GPSIMD)
  - aout/ns written interleaved into asn [128, 8] f32, cast to bf16
  - fold-in: ONE PE transpose [128,8] -> [8,128] psum (bf16) + evac
  - h1 = W1f @ feat (N=512) + W1as @ asT[2j:2j+2] (4 MMs N=128), accumulated
  - h2 = W2 @ h1s: 4 MMs N=512 (f32 psum)
  - w3 batch-major: lhsT = h2s[:, 128j:...] slices, rhs = w3 cols ->
    psum amlT [128, 4] directly batch-major (16 LDW+MM pairs, N=1)
  - a_ml = relu(psum + b3) fused in the ACT evac
  - dev@q_col / dev@g_col matvecs replaced by geometric recurrence
    s_t = ad_t + 0.25 s_{t-1} (cum_d = 2 ad + 0.375 s_prev; cum_dg = cg[t] s_t)
  - last step: only a_out is live; state/bgt/cum updates skipped
"""
import sys
import os

sys.path.insert(0, "/opt/trn_rl_repo")

import numpy as np
import ml_dtypes

D1, D2, D3 = 0.1, 1.0, 2.0
POWER = 10.0
STATE_CAP = 15.0
NCORES = 8

_CACHE = {}


def _scalars(H, lam, bud):
    t = np.arange(H)
    S = (1.0 - 0.25 ** (H - 1.0 - t)) / 0.75
    off = D1 / 8.0 * 10.0 + D2 / 4.0  # 0.375
    diag = 2.0 * D1 * 5.0 + D2  # 2.0
    gamma = (diag + off * S).astype(np.float32)
    cg = (off * S).astype(np.float32)
    inv_g = (1.0 / gamma.astype(np.float64)).astype(np.float32)
    lam32 = np.float32(lam)
    bud32 = np.float32(bud)
    per_step = np.float32(lam32 * np.float32(D3) + bud32 / np.float32(H))
    onelam = np.float32(np.float32(1.0) + lam32)
    econ = (lam32 * np.float32(D3)
            + (bud32 / np.float32(H)) * (t + 2.0).astype(np.float32)).astype(np.float32)
    return gamma, inv_g, cg, per_step, onelam, econ


def _build_program(H, lam, bud, b3v, mmdt_name):
    import concourse.tile as tile
    from concourse import bacc, mybir
    from contextlib import ExitStack

    f32 = mybir.dt.float32
    bf16 = mybir.dt.bfloat16
    mmdt = {"bf16": bf16, "f32": f32}[mmdt_name]
    Alu = mybir.AluOpType
    Act = mybir.ActivationFunctionType

    gamma, inv_g, cg, per_step, onelam, econ = _scalars(H, lam, bud)
    SQA = float(np.float32(np.sqrt(0.1)))
    SQB = float(np.float32(1.0 / (2.0 * np.sqrt(0.1))))
    gamma = [float(x) for x in gamma]
    inv_g = [float(x) for x in inv_g]
    cg = [float(x) for x in cg]
    econ = [float(x) for x in econ]
    per_step = float(per_step)
    onelam = float(onelam)

    nc = bacc.Bacc("TRN2", target_bir_lowering=False, debug=False,
                   enable_asserts=False)

    featT_d = nc.dram_tensor("featT", [H, 4, 1024], mmdt, kind="ExternalInput")
    nzd_d = nc.dram_tensor("nzd", [H, 128, 24], f32, kind="ExternalInput")
    as0_d = nc.dram_tensor("as0", [128, 16], f32, kind="ExternalInput")
    w1f_d = nc.dram_tensor("w1f", [4, 256], mmdt, kind="ExternalInput")
    w1as_d = nc.dram_tensor("w1as", [2, 256], mmdt, kind="ExternalInput")
    w1x_d = nc.dram_tensor("w1x", [6, 256], mmdt, kind="ExternalInput")
    w2a_d = nc.dram_tensor("w2a", [128, 256], mmdt, kind="ExternalInput")
    w2b_d = nc.dram_tensor("w2b", [128, 256], mmdt, kind="ExternalInput")
    w3_d = nc.dram_tensor("w3c", [128, 2], mmdt, kind="ExternalInput")
    b12_d = nc.dram_tensor("b12", [128, 4], f32, kind="ExternalInput")
    id_d = nc.dram_tensor("ident", [128, 128], mmdt, kind="ExternalInput")
    out_d = nc.dram_tensor("outb", [H, 128, 8], f32, kind="ExternalOutput")

    def mm(out, lhsT, rhs, **kw):
        nc.tensor.matmul(out, lhsT, rhs, **kw)

    with ExitStack() as ctx:
        tc = ctx.enter_context(tile.TileContext(nc))
        P = lambda name, bufs, **kw: ctx.enter_context(
            tc.tile_pool(name=name, bufs=bufs, **kw))

        consts = P("consts", 1)
        ftp = P("ftp", 3)
        nzp = P("nzp", 3)
        asnp = P("asnp", 3)     # asn f32 [128, 16] merged (a,s interleaved)
        asnbp = P("asnbp", 4)   # per-group bf16 [128, 8]
        x2p = P("x2p", 4)       # per-group x2 [2, 512]
        bgp = P("bgp", 3)
        ccp_ = P("ccp", 3)
        sap = P("sap", 3)
        h1p_ = P("h1sb", 3)
        h2p_ = P("h2sb", 3)
        tmp = P("tmp", 6)
        # PSUM banks: ph1 2 + ph2 2 + pml 2 (per-group) + pT 2 (per-group)
        ph1 = P("ph1", 2, space="PSUM")
        ph2 = P("ph2", 2, space="PSUM")
        pml = P("pml", 1, space="PSUM")
        pT = P("pT", 1, space="PSUM")
        phh = P("phh", 1, space="PSUM")

        w1f = consts.tile([4, 256], mmdt)
        nc.sync.dma_start(w1f[:], w1f_d.ap())
        w1as = consts.tile([2, 256], mmdt)
        nc.sync.dma_start(w1as[:], w1as_d.ap())
        w2a = consts.tile([128, 256], mmdt)
        nc.sync.dma_start(w2a[:], w2a_d.ap())
        w2b = consts.tile([128, 256], mmdt)
        nc.sync.dma_start(w2b[:], w2b_d.ap())
        w3 = consts.tile([128, 2], mmdt)
        nc.sync.dma_start(w3[:], w3_d.ap())
        b12 = consts.tile([128, 4], f32)
        nc.sync.dma_start(b12[:], b12_d.ap())
        ident = consts.tile([128, 128], mmdt)
        nc.sync.dma_start(ident[:], id_d.ap())
        psb = consts.tile([128, 1], f32)
        nc.vector.memset(psb[:], per_step)
        sqb = consts.tile([128, 1], f32)
        nc.vector.memset(sqb[:], SQB)
        w1x = consts.tile([6, 256], mmdt)
        nc.sync.dma_start(w1x[:], w1x_d.ap())

        as0sb = consts.tile([128, 16], f32)
        nc.sync.dma_start(as0sb[:], as0_d.ap())
        asn_prev = [None]
        a0i = asnp.tile([128, 16], f32, tag="asn", name="asn_init")
        nc.vector.tensor_copy(a0i[:], as0sb[:])
        asn_prev[0] = a0i
        bgt = [bgp.tile([128, 8], f32, tag="bg", name="bg_init")]
        cumc = [ccp_.tile([128, 8], f32, tag="cc", name="cc_init")]
        sacc = [sap.tile([128, 8], f32, tag="sa", name="sa_init")]
        nc.vector.memset(bgt[0][:], per_step)
        nc.gpsimd.memset(cumc[0][:], 0.0)
        nc.gpsimd.memset(sacc[0][:], 0.0)

        v, sc, gp, te = nc.vector, nc.scalar, nc.gpsimd, nc.tensor

        h1s_cur = [None, None]
        h2s_cur = [None, None]
        h1p_cur = [None, None]
        pml_cur = [None, None]
        x2_cur = [None, None]
        ft_cur = [None]
        nz_cur = [None]
        prep_ctx = [None, None]
        apm_cur = [None]

        ab_cur = [None, None]

        def fold_cast(g, t):
            ab = asnbp.tile([128, 8], bf16, tag=f"asnb{g}", name=f"asnb{g}_{t}")
            v.tensor_copy(ab[:], asn_prev[0][:, 8 * g:8 * g + 8])
            ab_cur[g] = ab

        def fold_T(g, t):
            ab = ab_cur[g]
            pt_ = pT.tile([2, 512], bf16, tag=f"pT{g}", name=f"pT{g}_{t}")
            for jj in range(4):
                te.transpose(pt_[:, 128 * jj:128 * (jj + 1)],
                             ab[:, 2 * jj:2 * jj + 2], ident[:])
            x6 = x2p.tile([6, 512], bf16, tag=f"x2{g}", name=f"x2{g}_{t}")
            nc.sync.dma_start(x6[2:6, :], featT_d.ap()[t, :, 512 * g:512 * (g + 1)])
            v.tensor_copy(x6[0:2, :], pt_[:])
            x2_cur[g] = x6

        def h1_as(g, t):
            x6 = x2_cur[g]
            h1s = []
            for mt in range(2):
                hp = ph1.tile([128, 512], f32, tag="h1", name=f"h1_{g}_{t}_{mt}")
                mm(hp[:], w1x[:, 128 * mt:128 * (mt + 1)], x6[:],
                   start=True, stop=True)
                hs = h1p_.tile([128, 512], mmdt, tag=f"h1s_{g}",
                               name=f"h1s_{g}_{t}_{mt}")
                if mt == 0:
                    sc.activation(hs[:], hp[:], Act.Relu, bias=b12[:, 0:1])
                else:
                    v.tensor_scalar(hs[:], hp[:], b12[:, 1:2], 0.0,
                                    op0=Alu.add, op1=Alu.max)
                h1s.append(hs)
            h1s_cur[g] = h1s

        h2p_cur = [None, None]

        def mlp_h2(g, t, ev1_inline=True):
            h1s = h1s_cur[g]
            h2s = []
            hps = []
            for mt in range(2):
                hp = ph2.tile([128, 512], f32, tag="h2", name=f"h2_{g}_{t}_{mt}")
                mm(hp[:], w2a[:, 128 * mt:128 * (mt + 1)], h1s[0][:],
                   start=True, stop=False)
                mm(hp[:], w2b[:, 128 * mt:128 * (mt + 1)], h1s[1][:],
                   start=False, stop=True)
                hps.append(hp)
                hs = h2p_.tile([128, 512], mmdt, tag=f"h2s_{g}",
                               name=f"h2s_{g}_{t}_{mt}")
                h2s.append(hs)
            sc.activation(h2s[0][:], hps[0][:], Act.Relu, bias=b12[:, 2:3])
            if ev1_inline:
                v.tensor_scalar(h2s[1][:], hps[1][:], b12[:, 3:4], 0.0,
                                op0=Alu.add, op1=Alu.max)
            h2p_cur[g] = hps
            h2s_cur[g] = h2s

        def h2_ev1(g, t):
            sc.activation(h2s_cur[g][1][:], h2p_cur[g][1][:], Act.Relu,
                          bias=b12[:, 3:4])

        def w3p0(g, t):
            h2s = h2s_cur[g]
            pm = pml.tile([128, 4], f32, tag=f"ml{g}", name=f"ml{g}_{t}")
            for jj in range(4):
                mm(pm[:, jj:jj + 1], h2s[0][:, 128 * jj:128 * (jj + 1)],
                   w3[:, 0:1], start=(jj == 0), stop=False)
            pml_cur[g] = pm

        def w3p1(g, t):
            h2s = h2s_cur[g]
            pm = pml_cur[g]
            for jj in range(4):
                mm(pm[:, jj:jj + 1], h2s[1][:, 128 * jj:128 * (jj + 1)],
                   w3[:, 1:2], start=False, stop=(jj == 3))

        def prep_m(t):
            """Pre-a_ml quantities for BOTH groups in one [128,8] pass on
            GPSIMD. Inputs (asn_prev, bgt, nz) are all ready right after
            state_tail(t-1), so this runs off the critical chain.
            apq = 0.8*a_prior = clip(s+d, 0, 8); consumers scale by 1.25
            (bit-exact vs. the old min(max(1.25*sd,0),10) form)."""
            nz = nz_cur[0]
            T8 = lambda tag: tmp.tile([128, 8], f32, tag=tag,
                                      name=f"{tag}_m{t}")
            dem = nz[:, 0:8]
            m2 = nz[:, 8:16]
            st_prev = asn_prev[0][:, 1:16:2]
            sd = T8("sdm")
            gp.tensor_tensor(sd[:], st_prev, dem, op=Alu.add)
            apq = apm_cur[0]
            gp.tensor_scalar(apq[:], sd[:], 0.0, POWER * 0.8,
                             op0=Alu.max, op1=Alu.min)
            c = T8("cm")
            gp.tensor_scalar_mul(c[:], bgt[0][:], inv_g[t])
            apn = T8("apn")
            gp.tensor_scalar_mul(apn[:], apq[:], 1.25)
            lo0 = T8("lo0m")
            gp.tensor_tensor(lo0[:], apn[:], c[:], op=Alu.subtract)
            gp.tensor_scalar(lo0[:], lo0[:], 0.0, None, op0=Alu.max)
            hi = T8("him")
            gp.tensor_tensor(hi[:], apn[:], c[:], op=Alu.add)
            z1 = T8("z1m")
            gp.tensor_tensor(z1[:], st_prev, m2, op=Alu.mult)
            z2 = T8("z2m")
            gp.tensor_tensor(z2[:], z1[:], dem, op=Alu.add)
            prep_ctx[0] = (lo0, hi, z2)

        def crit(g, t, asn):
            """Critical chain: t1 = max(pml+b3, lo0); aout = min(t1, hi);
            ns = clip(z2 - mn*aout, 0, 15)."""
            last = (t == H - 1)
            nz = nz_cur[0]
            lo0m, him, z2m = prep_ctx[0]
            sl = slice(4 * g, 4 * g + 4)
            T4 = lambda tag: tmp.tile([128, 4], f32, tag=f"{tag}{g}",
                                      name=f"{tag}{g}_{t}")
            mn = nz[:, 16 + 4 * g:20 + 4 * g]
            t1 = T4("t1")
            v.scalar_tensor_tensor(t1[:], pml_cur[g][:], b3v, lo0m[:, sl],
                                   op0=Alu.add, op1=Alu.max)
            aout = asn[:, 8 * g:8 * g + 8:2]
            v.tensor_tensor(aout, t1[:], him[:, sl], op=Alu.min)
            if not last:
                z3 = T4("z3")
                v.tensor_tensor(z3[:], mn, aout, op=Alu.mult)
                z4 = T4("z4")
                v.tensor_tensor(z4[:], z2m[:, sl], z3[:], op=Alu.subtract)
                ns = asn[:, 8 * g + 1:8 * g + 8:2]
                v.tensor_scalar(ns, z4[:], 0.0, STATE_CAP, op0=Alu.max, op1=Alu.min)

        def state_tail(t, asn):
            if t == H - 1:
                return
            apq = apm_cur[0]
            T8 = lambda tag: tmp.tile([128, 8], f32, tag=tag, name=f"{tag}_{t}")
            aout = asn[:, 0:16:2]
            ns = asn[:, 1:16:2]
            # dd = aout - ap where ap = 1.25*apq (bit-exact vs old form)
            dd = T8("dd")
            v.scalar_tensor_tensor(dd[:], apq[:], -1.25, aout,
                                   op0=Alu.mult, op1=Alu.add)
            ad = T8("ad")
            sc.activation(ad[:], dd[:], Act.Abs)
            # c_cost = D1*ns^2 + D2*ns + D3 = Square(sqrt(D1)*ns + D2/(2 sqrt(D1)))
            #          + (D3 - D2^2/(4 D1)) = cc' - 0.5
            # Shift propagates: u1' = u1+0.5, u2' = u2+0.5,
            # cp1'' = max(u2',2.5)*onelam = cp1 + 0.5*onelam, q2' = q2+0.5*onelam
            # ccn = q2 - c_cost = (q2' - 0.5*onelam) - (cc' - 0.5)
            #     = (q2' + (0.5 - 0.5*onelam)) - cc'
            ccp = T8("ccp")
            sc.activation(ccp[:], ns, Act.Square, bias=sqb[:, 0:1], scale=SQA)
            u1 = T8("u1")
            v.scalar_tensor_tensor(u1[:], ad[:], -2.0, ccp[:],
                                   op0=Alu.mult, op1=Alu.add)
            u2 = T8("u2")
            v.scalar_tensor_tensor(u2[:], sacc[0][:], -0.375, u1[:],
                                   op0=Alu.mult, op1=Alu.add)
            sn = sap.tile([128, 8], f32, tag="sa", name=f"sa_{t}")
            v.scalar_tensor_tensor(sn[:], sacc[0][:], 0.25, ad[:],
                                   op0=Alu.mult, op1=Alu.add)
            cp1 = T8("cp1")
            v.tensor_scalar(cp1[:], u2[:], 2.5, onelam, op0=Alu.max, op1=Alu.mult)
            q2 = T8("q2")
            gp.tensor_tensor(q2[:], cumc[0][:], cp1[:], op=Alu.add)
            ccn = ccp_.tile([128, 8], f32, tag="cc", name=f"cc_{t}")
            v.scalar_tensor_tensor(ccn[:], q2[:], float(0.5 - 0.5 * onelam),
                                   ccp[:], op0=Alu.add, op1=Alu.subtract)
            v1 = T8("v1")
            v.scalar_tensor_tensor(v1[:], ad[:], -gamma[t], bgt[0][:],
                                   op0=Alu.mult, op1=Alu.add)
            e1 = T8("e1")
            sc.activation(e1[:], v1[:], Act.Relu, bias=psb[:, 0:1])
            v2 = T8("v2")
            v.scalar_tensor_tensor(v2[:], sn[:], -cg[t], ccn[:],
                                   op0=Alu.mult, op1=Alu.add)
            bn = bgp.tile([128, 8], f32, tag="bg", name=f"bg_{t}")
            v.scalar_tensor_tensor(bn[:], v2[:], econ[t], e1[:],
                                   op0=Alu.add, op1=Alu.max)
            bgt[0] = bn
            cumc[0] = ccn
            sacc[0] = sn

        NHEAT = int(os.environ.get("KHEAT", "0"))
        NWARM = int(os.environ.get("KWARM", "0"))

        def heat(tag, t, n):
            if n <= 0:
                return
            for i in range(n):
                htile = phh.tile([128, 256], f32, tag="hh", name=f"heat_{tag}_{t}_{i}")
                mm(htile[:], ident[:], w2a[:, 0:256],
                   start=True, stop=True)

        def load_inputs(t):
            nz = nzp.tile([128, 24], f32, tag="nz", name=f"nz_{t}")
            nc.sync.dma_start(nz[:], nzd_d.ap()[t])
            return None, nz

        ft0, nz0 = load_inputs(0)
        if NWARM:
            for i in range(NWARM):
                wp = ph1.tile([128, 256], f32, tag="h1", name=f"warm_{i}")
                mm(wp[:], w2a[:, 0:128], w2b[:, 0:256], start=True, stop=True)
        fold_cast(0, 0)
        fold_cast(1, 0)
        fold_T(0, 0)
        nz_t = nz0

        for t in range(H):
            nz_cur[0] = nz_t
            apm_cur[0] = tmp.tile([128, 8], f32, tag="apm", name=f"apm_{t}")
            asn = asnp.tile([128, 16], f32, tag="asn", name=f"asn_{t}")
            h1_as(0, t)
            fold_T(1, t)
            prep_m(t)
            h1_as(1, t)
            mlp_h2(0, t, ev1_inline=True)
            mlp_h2(1, t, ev1_inline=False)
            w3p0(0, t)
            if t + 1 < H:
                ftn, nzn = load_inputs(t + 1)
                ft_cur[0] = ftn
            w3p0(1, t)
            w3p1(0, t)
            crit(0, t, asn)
            asn_prev[0] = asn
            if t + 1 < H:
                fold_cast(0, t + 1)
            h2_ev1(1, t)
            heat("pT0", t, NHEAT)
            if t + 1 < H:
                fold_T(0, t + 1)
            w3p1(1, t)
            crit(1, t, asn)
            if t + 1 < H:
                fold_cast(1, t + 1)
            state_tail(t, asn)
            nc.sync.dma_start(out_d.ap()[t], asn[:, 0:16:2])
            if t + 1 < H:
                nz_t = nzn

    nc.compile()
    return nc


def _prep_core(pc, tn, dn, ap_, sp_, mmdt_name):
    """Per-core input arrays. pc: [1024, 96, 4]; tn/dn: [1024, 96]; ap_/sp_: [1024]."""
    H = pc.shape[1]
    mmnp = ml_dtypes.bfloat16 if mmdt_name == "bf16" else np.float32
    featT = np.ascontiguousarray(pc.transpose(1, 2, 0)).astype(mmnp)
    def bm(a):  # [1024, H] -> [H, 128, 8]
        return np.ascontiguousarray(
            a.reshape(2, 4, 128, H).transpose(3, 2, 0, 1).reshape(H, 128, 8))
    dem = bm(pc[:, :, 0])
    m2 = bm(1.0 - dn)
    mn = bm(0.8 + tn)
    nzd = np.ascontiguousarray(
        np.concatenate([dem, m2, mn], axis=2)).astype(np.float32)
    a = ap_.reshape(2, 4, 128)
    s = sp_.reshape(2, 4, 128)
    as0 = np.ascontiguousarray(
        np.stack([a, s], -1).transpose(2, 0, 1, 3).reshape(128, 16)).astype(np.float32)
    return featT, nzd, as0


def _prepare(policy_in_c, trans_noise, demand_noise, action_pre, state_pre,
             Lambda, Budget, W1, b1, W2, b2, W3, b3):
    mmdt_name = os.environ.get("KBASS_DT", "bf16")
    pc = np.asarray(policy_in_c, np.float32)
    tn = np.asarray(trans_noise, np.float32)[..., 0]
    dn = np.asarray(demand_noise, np.float32)[..., 0]
    ap_ = np.asarray(action_pre, np.float32)[:, 0]
    sp_ = np.asarray(state_pre, np.float32)[:, 0]
    lam = float(np.asarray(Lambda, np.float32)[0])
    bud = float(np.asarray(Budget, np.float32)[0])
    W1 = np.asarray(W1, np.float32)
    b1 = np.asarray(b1, np.float32)
    W2 = np.asarray(W2, np.float32)
    b2 = np.asarray(b2, np.float32)
    W3 = np.asarray(W3, np.float32)
    b3v = float(np.asarray(b3, np.float32)[0])
    B, H, C = pc.shape
    Bc = B // NCORES

    key = (H, lam, bud, b3v, mmdt_name, "v2")
    if key not in _CACHE:
        _CACHE[key] = _build_program(H, lam, bud, b3v, mmdt_name)
    nc = _CACHE[key]

    mmnp = ml_dtypes.bfloat16 if mmdt_name == "bf16" else np.float32
    # x rows: [action, state, feat0..3]; reference x = [feat, action, state]
    # so W1 cols 0:4 = feat, col 4 = action, col 5 = state.
    w1f = np.ascontiguousarray(W1[:, 0:4].T).astype(mmnp)     # [4, 256]
    w1as = np.ascontiguousarray(W1[:, 4:6].T).astype(mmnp)    # [2, 256]
    w1x = np.ascontiguousarray(W1[:, [4, 5, 0, 1, 2, 3]].T).astype(mmnp)  # [6, 256]
    w2t = np.ascontiguousarray(W2.T).astype(mmnp)
    w2a_np, w2b_np = w2t[0:128], w2t[128:256]
    w3c = np.ascontiguousarray(np.stack([W3[0, 0:128], W3[0, 128:256]], 1)).astype(mmnp)
    b12 = np.ascontiguousarray(
        np.stack([b1[0:128], b1[128:256], b2[0:128], b2[128:256]], 1)).astype(np.float32)
    ident = np.eye(128, dtype=np.float32).astype(mmnp)

    in_maps = []
    for cid in range(NCORES):
        sl = slice(cid * Bc, (cid + 1) * Bc)
        featT, nzd, as0 = _prep_core(pc[sl], tn[sl], dn[sl], ap_[sl], sp_[sl],
                                     mmdt_name)
        in_maps.append({
            "featT": featT, "nzd": nzd, "as0": as0,
            "w1f": w1f, "w1as": w1as, "w1x": w1x, "w2a": w2a_np, "w2b": w2b_np,
            "w3c": w3c, "b12": b12, "ident": ident,
        })

    return nc, in_maps, B, H, Bc


def _assemble(res, B, H, Bc):
    out = np.empty((B, H), np.float32)
    for cid in range(NCORES):
        ob = res.results[cid]["outb"]  # [H, 128, 8]
        oc = ob.reshape(H, 128, 2, 4).transpose(2, 3, 1, 0).reshape(Bc, H)
        out[cid * Bc:(cid + 1) * Bc] = oc
    return out


def kernel(**inputs):
    from concourse.bass_utils import run_bass_kernel_spmd
    nc, in_maps, B, H, Bc = _prepare(**inputs)
    res = run_bass_kernel_spmd(nc, in_maps, core_ids=list(range(NCORES)))
    return _assemble(res, B, H, Bc)



# revision 13
# speedup vs baseline: 1.1881x; 1.0267x over previous
"""Trainium2 Bass kernel for nn_Net_3659312136203 — v2.

Data-parallel over batch (8192 -> 8 cores x 1024). Per core, 96-step scan
with two independent 512-row groups software-pipelined so the PE never
starves (HAM stays at K=8/8).

Per step, per group g (batch blocks j=0..3, 128 rows each):
  - state math batch-major on [128, 4] tiles (DVE/ACT/GPSIMD)
  - aout/ns written interleaved into asn [128, 8] f32, cast to bf16
  - fold-in: ONE PE transpose [128,8] -> [8,128] psum (bf16) + evac
  - h1 = W1f @ feat (N=512) + W1as @ asT[2j:2j+2] (4 MMs N=128), accumulated
  - h2 = W2 @ h1s: 4 MMs N=512 (f32 psum)
  - w3 batch-major: lhsT = h2s[:, 128j:...] slices, rhs = w3 cols ->
    psum amlT [128, 4] directly batch-major (16 LDW+MM pairs, N=1)
  - a_ml = relu(psum + b3) fused in the ACT evac
  - dev@q_col / dev@g_col matvecs replaced by geometric recurrence
    s_t = ad_t + 0.25 s_{t-1} (cum_d = 2 ad + 0.375 s_prev; cum_dg = cg[t] s_t)
  - last step: only a_out is live; state/bgt/cum updates skipped
"""
import sys
import os

sys.path.insert(0, "/opt/trn_rl_repo")

import numpy as np
import ml_dtypes

D1, D2, D3 = 0.1, 1.0, 2.0
POWER = 10.0
STATE_CAP = 15.0
NCORES = 8

_CACHE = {}


def _scalars(H, lam, bud):
    t = np.arange(H)
    S = (1.0 - 0.25 ** (H - 1.0 - t)) / 0.75
    off = D1 / 8.0 * 10.0 + D2 / 4.0  # 0.375
    diag = 2.0 * D1 * 5.0 + D2  # 2.0
    gamma = (diag + off * S).astype(np.float32)
    cg = (off * S).astype(np.float32)
    inv_g = (1.0 / gamma.astype(np.float64)).astype(np.float32)
    lam32 = np.float32(lam)
    bud32 = np.float32(bud)
    per_step = np.float32(lam32 * np.float32(D3) + bud32 / np.float32(H))
    onelam = np.float32(np.float32(1.0) + lam32)
    econ = (lam32 * np.float32(D3)
            + (bud32 / np.float32(H)) * (t + 2.0).astype(np.float32)).astype(np.float32)
    return gamma, inv_g, cg, per_step, onelam, econ


def _build_program(H, lam, bud, b3v, mmdt_name):
    import concourse.tile as tile
    from concourse import bacc, mybir
    from contextlib import ExitStack

    f32 = mybir.dt.float32
    bf16 = mybir.dt.bfloat16
    mmdt = {"bf16": bf16, "f32": f32}[mmdt_name]
    Alu = mybir.AluOpType
    Act = mybir.ActivationFunctionType

    gamma, inv_g, cg, per_step, onelam, econ = _scalars(H, lam, bud)
    SQA = float(np.float32(np.sqrt(0.1)))
    SQB = float(np.float32(1.0 / (2.0 * np.sqrt(0.1))))
    gamma = [float(x) for x in gamma]
    inv_g = [float(x) for x in inv_g]
    cg = [float(x) for x in cg]
    econ = [float(x) for x in econ]
    per_step = float(per_step)
    onelam = float(onelam)

    nc = bacc.Bacc("TRN2", target_bir_lowering=False, debug=False,
                   enable_asserts=False)

    featT_d = nc.dram_tensor("featT", [H, 4, 1024], mmdt, kind="ExternalInput")
    nzd_d = nc.dram_tensor("nzd", [H, 128, 24], f32, kind="ExternalInput")
    as0_d = nc.dram_tensor("as0", [128, 16], f32, kind="ExternalInput")
    w1f_d = nc.dram_tensor("w1f", [4, 256], mmdt, kind="ExternalInput")
    w1as_d = nc.dram_tensor("w1as", [2, 256], mmdt, kind="ExternalInput")
    w1x_d = nc.dram_tensor("w1x", [6, 256], mmdt, kind="ExternalInput")
    w2a_d = nc.dram_tensor("w2a", [128, 256], mmdt, kind="ExternalInput")
    w2b_d = nc.dram_tensor("w2b", [128, 256], mmdt, kind="ExternalInput")
    w3_d = nc.dram_tensor("w3c", [128, 2], mmdt, kind="ExternalInput")
    b12_d = nc.dram_tensor("b12", [128, 4], f32, kind="ExternalInput")
    id_d = nc.dram_tensor("ident", [128, 128], mmdt, kind="ExternalInput")
    out_d = nc.dram_tensor("outb", [H, 128, 8], f32, kind="ExternalOutput")

    def mm(out, lhsT, rhs, **kw):
        nc.tensor.matmul(out, lhsT, rhs, **kw)

    with ExitStack() as ctx:
        tc = ctx.enter_context(tile.TileContext(nc))
        P = lambda name, bufs, **kw: ctx.enter_context(
            tc.tile_pool(name=name, bufs=bufs, **kw))

        consts = P("consts", 1)
        ftp = P("ftp", 3)
        nzp = P("nzp", 3)
        asnp = P("asnp", 3)     # asn f32 [128, 16] merged (a,s interleaved)
        asnbp = P("asnbp", 4)   # per-group bf16 [128, 8]
        x2p = P("x2p", 4)       # per-group x2 [2, 512]
        bgp = P("bgp", 3)
        ccp_ = P("ccp", 3)
        sap = P("sap", 3)
        h1p_ = P("h1sb", 3)
        h2p_ = P("h2sb", 3)
        tmp = P("tmp", 6)
        # PSUM banks: ph1 2 + ph2 2 + pml 2 (per-group) + pT 2 (per-group)
        ph1 = P("ph1", 2, space="PSUM")
        ph2 = P("ph2", 2, space="PSUM")
        pml = P("pml", 1, space="PSUM")
        pT = P("pT", 1, space="PSUM")
        phh = P("phh", 1, space="PSUM")

        w1f = consts.tile([4, 256], mmdt)
        nc.sync.dma_start(w1f[:], w1f_d.ap())
        w1as = consts.tile([2, 256], mmdt)
        nc.sync.dma_start(w1as[:], w1as_d.ap())
        w2a = consts.tile([128, 256], mmdt)
        nc.sync.dma_start(w2a[:], w2a_d.ap())
        w2b = consts.tile([128, 256], mmdt)
        nc.sync.dma_start(w2b[:], w2b_d.ap())
        w3 = consts.tile([128, 2], mmdt)
        nc.sync.dma_start(w3[:], w3_d.ap())
        b12 = consts.tile([128, 4], f32)
        nc.sync.dma_start(b12[:], b12_d.ap())
        ident = consts.tile([128, 128], mmdt)
        nc.sync.dma_start(ident[:], id_d.ap())
        psb = consts.tile([128, 1], f32)
        nc.vector.memset(psb[:], per_step)
        sqb = consts.tile([128, 1], f32)
        nc.vector.memset(sqb[:], SQB)
        w1x = consts.tile([6, 256], mmdt)
        nc.sync.dma_start(w1x[:], w1x_d.ap())

        as0sb = consts.tile([128, 16], f32)
        nc.sync.dma_start(as0sb[:], as0_d.ap())
        asn_prev = [None]
        a0i = asnp.tile([128, 16], f32, tag="asn", name="asn_init")
        nc.vector.tensor_copy(a0i[:], as0sb[:])
        asn_prev[0] = a0i
        bgt = [bgp.tile([128, 8], f32, tag="bg", name="bg_init")]
        cumc = [ccp_.tile([128, 8], f32, tag="cc", name="cc_init")]
        sacc = [sap.tile([128, 8], f32, tag="sa", name="sa_init")]
        nc.vector.memset(bgt[0][:], per_step)
        nc.gpsimd.memset(cumc[0][:], 0.0)
        nc.gpsimd.memset(sacc[0][:], 0.0)

        v, sc, gp, te = nc.vector, nc.scalar, nc.gpsimd, nc.tensor

        h1s_cur = [None, None]
        h2s_cur = [None, None]
        h1p_cur = [None, None]
        pml_cur = [None, None]
        x2_cur = [None, None]
        ft_cur = [None]
        nz_cur = [None]
        prep_ctx = [None, None]
        apm_cur = [None]

        ab_cur = [None, None]

        def fold_cast(g, t):
            ab = asnbp.tile([128, 8], bf16, tag=f"asnb{g}", name=f"asnb{g}_{t}")
            v.tensor_copy(ab[:], asn_prev[0][:, 8 * g:8 * g + 8])
            ab_cur[g] = ab

        def fold_T(g, t):
            ab = ab_cur[g]
            pt_ = pT.tile([2, 512], bf16, tag=f"pT{g}", name=f"pT{g}_{t}")
            for jj in range(4):
                te.transpose(pt_[:, 128 * jj:128 * (jj + 1)],
                             ab[:, 2 * jj:2 * jj + 2], ident[:])
            x6 = x2p.tile([6, 512], bf16, tag=f"x2{g}", name=f"x2{g}_{t}")
            nc.sync.dma_start(x6[2:6, :], featT_d.ap()[t, :, 512 * g:512 * (g + 1)])
            v.tensor_copy(x6[0:2, :], pt_[:])
            x2_cur[g] = x6

        def h1_as(g, t):
            x6 = x2_cur[g]
            h1s = []
            for mt in range(2):
                hp = ph1.tile([128, 512], f32, tag="h1", name=f"h1_{g}_{t}_{mt}")
                mm(hp[:], w1x[:, 128 * mt:128 * (mt + 1)], x6[:],
                   start=True, stop=True)
                hs = h1p_.tile([128, 512], mmdt, tag=f"h1s_{g}",
                               name=f"h1s_{g}_{t}_{mt}")
                if mt == 0:
                    sc.activation(hs[:], hp[:], Act.Relu, bias=b12[:, 0:1])
                else:
                    v.tensor_scalar(hs[:], hp[:], b12[:, 1:2], 0.0,
                                    op0=Alu.add, op1=Alu.max)
                h1s.append(hs)
            h1s_cur[g] = h1s

        h2p_cur = [None, None]

        def mlp_h2(g, t, ev1_inline=True):
            h1s = h1s_cur[g]
            h2s = []
            hps = []
            for mt in range(2):
                hp = ph2.tile([128, 512], f32, tag="h2", name=f"h2_{g}_{t}_{mt}")
                mm(hp[:], w2a[:, 128 * mt:128 * (mt + 1)], h1s[0][:],
                   start=True, stop=False)
                mm(hp[:], w2b[:, 128 * mt:128 * (mt + 1)], h1s[1][:],
                   start=False, stop=True)
                hps.append(hp)
                hs = h2p_.tile([128, 512], mmdt, tag=f"h2s_{g}",
                               name=f"h2s_{g}_{t}_{mt}")
                h2s.append(hs)
            for half in range(2):
                cs = slice(256 * half, 256 * half + 256)
                sc.activation(h2s[0][:, cs], hps[0][:, cs], Act.Relu,
                              bias=b12[:, 2:3])
            if ev1_inline:
                for half in range(2):
                    cs = slice(256 * half, 256 * half + 256)
                    v.tensor_scalar(h2s[1][:, cs], hps[1][:, cs], b12[:, 3:4],
                                    0.0, op0=Alu.add, op1=Alu.max)
            h2p_cur[g] = hps
            h2s_cur[g] = h2s

        def h2_ev1(g, t):
            for half in range(2):
                cs = slice(256 * half, 256 * half + 256)
                sc.activation(h2s_cur[g][1][:, cs], h2p_cur[g][1][:, cs],
                              Act.Relu, bias=b12[:, 3:4])

        def w3p0(g, t):
            h2s = h2s_cur[g]
            pm = pml.tile([128, 4], f32, tag=f"ml{g}", name=f"ml{g}_{t}")
            for jj in range(4):
                mm(pm[:, jj:jj + 1], h2s[0][:, 128 * jj:128 * (jj + 1)],
                   w3[:, 0:1], start=(jj == 0), stop=False)
            pml_cur[g] = pm

        def w3p1(g, t):
            h2s = h2s_cur[g]
            pm = pml_cur[g]
            for jj in range(4):
                mm(pm[:, jj:jj + 1], h2s[1][:, 128 * jj:128 * (jj + 1)],
                   w3[:, 1:2], start=False, stop=(jj == 3))

        def prep_m(t):
            """Pre-a_ml quantities for BOTH groups in one [128,8] pass on
            GPSIMD. Inputs (asn_prev, bgt, nz) are all ready right after
            state_tail(t-1), so this runs off the critical chain.
            apq = 0.8*a_prior = clip(s+d, 0, 8); consumers scale by 1.25
            (bit-exact vs. the old min(max(1.25*sd,0),10) form)."""
            nz = nz_cur[0]
            T8 = lambda tag: tmp.tile([128, 8], f32, tag=tag,
                                      name=f"{tag}_m{t}")
            dem = nz[:, 0:8]
            m2 = nz[:, 8:16]
            st_prev = asn_prev[0][:, 1:16:2]
            sd = T8("sdm")
            gp.tensor_tensor(sd[:], st_prev, dem, op=Alu.add)
            apq = apm_cur[0]
            gp.tensor_scalar(apq[:], sd[:], 0.0, POWER * 0.8,
                             op0=Alu.max, op1=Alu.min)
            c = T8("cm")
            gp.tensor_scalar_mul(c[:], bgt[0][:], inv_g[t])
            apn = T8("apn")
            gp.tensor_scalar_mul(apn[:], apq[:], 1.25)
            lo0 = T8("lo0m")
            gp.tensor_tensor(lo0[:], apn[:], c[:], op=Alu.subtract)
            gp.tensor_scalar(lo0[:], lo0[:], 0.0, None, op0=Alu.max)
            hi = T8("him")
            gp.tensor_tensor(hi[:], apn[:], c[:], op=Alu.add)
            z1 = T8("z1m")
            gp.tensor_tensor(z1[:], st_prev, m2, op=Alu.mult)
            z2 = T8("z2m")
            gp.tensor_tensor(z2[:], z1[:], dem, op=Alu.add)
            prep_ctx[0] = (lo0, hi, z2)

        def crit(g, t, asn):
            """Critical chain: t1 = max(pml+b3, lo0); aout = min(t1, hi);
            ns = clip(z2 - mn*aout, 0, 15)."""
            last = (t == H - 1)
            nz = nz_cur[0]
            lo0m, him, z2m = prep_ctx[0]
            sl = slice(4 * g, 4 * g + 4)
            T4 = lambda tag: tmp.tile([128, 4], f32, tag=f"{tag}{g}",
                                      name=f"{tag}{g}_{t}")
            mn = nz[:, 16 + 4 * g:20 + 4 * g]
            t1 = T4("t1")
            v.scalar_tensor_tensor(t1[:], pml_cur[g][:], b3v, lo0m[:, sl],
                                   op0=Alu.add, op1=Alu.max)
            aout = asn[:, 8 * g:8 * g + 8:2]
            v.tensor_tensor(aout, t1[:], him[:, sl], op=Alu.min)
            if not last:
                z3 = T4("z3")
                v.tensor_tensor(z3[:], mn, aout, op=Alu.mult)
                z4 = T4("z4")
                v.tensor_tensor(z4[:], z2m[:, sl], z3[:], op=Alu.subtract)
                ns = asn[:, 8 * g + 1:8 * g + 8:2]
                v.tensor_scalar(ns, z4[:], 0.0, STATE_CAP, op0=Alu.max, op1=Alu.min)

        def state_tail(t, asn):
            if t == H - 1:
                return
            apq = apm_cur[0]
            T8 = lambda tag: tmp.tile([128, 8], f32, tag=tag, name=f"{tag}_{t}")
            aout = asn[:, 0:16:2]
            ns = asn[:, 1:16:2]
            # dd = aout - ap where ap = 1.25*apq (bit-exact vs old form)
            dd = T8("dd")
            v.scalar_tensor_tensor(dd[:], apq[:], -1.25, aout,
                                   op0=Alu.mult, op1=Alu.add)
            ad = T8("ad")
            sc.activation(ad[:], dd[:], Act.Abs)
            # c_cost = D1*ns^2 + D2*ns + D3 = Square(sqrt(D1)*ns + D2/(2 sqrt(D1)))
            #          + (D3 - D2^2/(4 D1)) = cc' - 0.5
            # Shift propagates: u1' = u1+0.5, u2' = u2+0.5,
            # cp1'' = max(u2',2.5)*onelam = cp1 + 0.5*onelam, q2' = q2+0.5*onelam
            # ccn = q2 - c_cost = (q2' - 0.5*onelam) - (cc' - 0.5)
            #     = (q2' + (0.5 - 0.5*onelam)) - cc'
            ccp = T8("ccp")
            sc.activation(ccp[:], ns, Act.Square, bias=sqb[:, 0:1], scale=SQA)
            u1 = T8("u1")
            v.scalar_tensor_tensor(u1[:], ad[:], -2.0, ccp[:],
                                   op0=Alu.mult, op1=Alu.add)
            u2 = T8("u2")
            v.scalar_tensor_tensor(u2[:], sacc[0][:], -0.375, u1[:],
                                   op0=Alu.mult, op1=Alu.add)
            sn = sap.tile([128, 8], f32, tag="sa", name=f"sa_{t}")
            v.scalar_tensor_tensor(sn[:], sacc[0][:], 0.25, ad[:],
                                   op0=Alu.mult, op1=Alu.add)
            cp1 = T8("cp1")
            v.tensor_scalar(cp1[:], u2[:], 2.5, onelam, op0=Alu.max, op1=Alu.mult)
            q2 = T8("q2")
            gp.tensor_tensor(q2[:], cumc[0][:], cp1[:], op=Alu.add)
            ccn = ccp_.tile([128, 8], f32, tag="cc", name=f"cc_{t}")
            v.scalar_tensor_tensor(ccn[:], q2[:], float(0.5 - 0.5 * onelam),
                                   ccp[:], op0=Alu.add, op1=Alu.subtract)
            v1 = T8("v1")
            v.scalar_tensor_tensor(v1[:], ad[:], -gamma[t], bgt[0][:],
                                   op0=Alu.mult, op1=Alu.add)
            e1 = T8("e1")
            sc.activation(e1[:], v1[:], Act.Relu, bias=psb[:, 0:1])
            v2 = T8("v2")
            v.scalar_tensor_tensor(v2[:], sn[:], -cg[t], ccn[:],
                                   op0=Alu.mult, op1=Alu.add)
            bn = bgp.tile([128, 8], f32, tag="bg", name=f"bg_{t}")
            v.scalar_tensor_tensor(bn[:], v2[:], econ[t], e1[:],
                                   op0=Alu.add, op1=Alu.max)
            bgt[0] = bn
            cumc[0] = ccn
            sacc[0] = sn

        NHEAT = int(os.environ.get("KHEAT", "0"))
        NWARM = int(os.environ.get("KWARM", "0"))

        def heat(tag, t, n):
            if n <= 0:
                return
            for i in range(n):
                htile = phh.tile([128, 256], f32, tag="hh", name=f"heat_{tag}_{t}_{i}")
                mm(htile[:], ident[:], w2a[:, 0:256],
                   start=True, stop=True)

        def load_inputs(t):
            nz = nzp.tile([128, 24], f32, tag="nz", name=f"nz_{t}")
            nc.sync.dma_start(nz[:], nzd_d.ap()[t])
            return None, nz

        ft0, nz0 = load_inputs(0)
        if NWARM:
            for i in range(NWARM):
                wp = ph1.tile([128, 256], f32, tag="h1", name=f"warm_{i}")
                mm(wp[:], w2a[:, 0:128], w2b[:, 0:256], start=True, stop=True)
        fold_cast(0, 0)
        fold_cast(1, 0)
        fold_T(0, 0)
        nz_t = nz0

        for t in range(H):
            nz_cur[0] = nz_t
            apm_cur[0] = tmp.tile([128, 8], f32, tag="apm", name=f"apm_{t}")
            asn = asnp.tile([128, 16], f32, tag="asn", name=f"asn_{t}")
            h1_as(0, t)
            fold_T(1, t)
            prep_m(t)
            h1_as(1, t)
            mlp_h2(0, t, ev1_inline=True)
            mlp_h2(1, t, ev1_inline=False)
            w3p0(0, t)
            if t + 1 < H:
                ftn, nzn = load_inputs(t + 1)
                ft_cur[0] = ftn
            w3p0(1, t)
            w3p1(0, t)
            crit(0, t, asn)
            asn_prev[0] = asn
            if t + 1 < H:
                fold_cast(0, t + 1)
            h2_ev1(1, t)
            heat("pT0", t, NHEAT)
            if t + 1 < H:
                fold_T(0, t + 1)
            w3p1(1, t)
            crit(1, t, asn)
            if t + 1 < H:
                fold_cast(1, t + 1)
            state_tail(t, asn)
            nc.sync.dma_start(out_d.ap()[t], asn[:, 0:16:2])
            if t + 1 < H:
                nz_t = nzn

    nc.compile()
    return nc


def _prep_core(pc, tn, dn, ap_, sp_, mmdt_name):
    """Per-core input arrays. pc: [1024, 96, 4]; tn/dn: [1024, 96]; ap_/sp_: [1024]."""
    H = pc.shape[1]
    mmnp = ml_dtypes.bfloat16 if mmdt_name == "bf16" else np.float32
    featT = np.ascontiguousarray(pc.transpose(1, 2, 0)).astype(mmnp)
    def bm(a):  # [1024, H] -> [H, 128, 8]
        return np.ascontiguousarray(
            a.reshape(2, 4, 128, H).transpose(3, 2, 0, 1).reshape(H, 128, 8))
    dem = bm(pc[:, :, 0])
    m2 = bm(1.0 - dn)
    mn = bm(0.8 + tn)
    nzd = np.ascontiguousarray(
        np.concatenate([dem, m2, mn], axis=2)).astype(np.float32)
    a = ap_.reshape(2, 4, 128)
    s = sp_.reshape(2, 4, 128)
    as0 = np.ascontiguousarray(
        np.stack([a, s], -1).transpose(2, 0, 1, 3).reshape(128, 16)).astype(np.float32)
    return featT, nzd, as0


def _prepare(policy_in_c, trans_noise, demand_noise, action_pre, state_pre,
             Lambda, Budget, W1, b1, W2, b2, W3, b3):
    mmdt_name = os.environ.get("KBASS_DT", "bf16")
    pc = np.asarray(policy_in_c, np.float32)
    tn = np.asarray(trans_noise, np.float32)[..., 0]
    dn = np.asarray(demand_noise, np.float32)[..., 0]
    ap_ = np.asarray(action_pre, np.float32)[:, 0]
    sp_ = np.asarray(state_pre, np.float32)[:, 0]
    lam = float(np.asarray(Lambda, np.float32)[0])
    bud = float(np.asarray(Budget, np.float32)[0])
    W1 = np.asarray(W1, np.float32)
    b1 = np.asarray(b1, np.float32)
    W2 = np.asarray(W2, np.float32)
    b2 = np.asarray(b2, np.float32)
    W3 = np.asarray(W3, np.float32)
    b3v = float(np.asarray(b3, np.float32)[0])
    B, H, C = pc.shape
    Bc = B // NCORES

    key = (H, lam, bud, b3v, mmdt_name, "v2")
    if key not in _CACHE:
        _CACHE[key] = _build_program(H, lam, bud, b3v, mmdt_name)
    nc = _CACHE[key]

    mmnp = ml_dtypes.bfloat16 if mmdt_name == "bf16" else np.float32
    # x rows: [action, state, feat0..3]; reference x = [feat, action, state]
    # so W1 cols 0:4 = feat, col 4 = action, col 5 = state.
    w1f = np.ascontiguousarray(W1[:, 0:4].T).astype(mmnp)     # [4, 256]
    w1as = np.ascontiguousarray(W1[:, 4:6].T).astype(mmnp)    # [2, 256]
    w1x = np.ascontiguousarray(W1[:, [4, 5, 0, 1, 2, 3]].T).astype(mmnp)  # [6, 256]
    w2t = np.ascontiguousarray(W2.T).astype(mmnp)
    w2a_np, w2b_np = w2t[0:128], w2t[128:256]
    w3c = np.ascontiguousarray(np.stack([W3[0, 0:128], W3[0, 128:256]], 1)).astype(mmnp)
    b12 = np.ascontiguousarray(
        np.stack([b1[0:128], b1[128:256], b2[0:128], b2[128:256]], 1)).astype(np.float32)
    ident = np.eye(128, dtype=np.float32).astype(mmnp)

    in_maps = []
    for cid in range(NCORES):
        sl = slice(cid * Bc, (cid + 1) * Bc)
        featT, nzd, as0 = _prep_core(pc[sl], tn[sl], dn[sl], ap_[sl], sp_[sl],
                                     mmdt_name)
        in_maps.append({
            "featT": featT, "nzd": nzd, "as0": as0,
            "w1f": w1f, "w1as": w1as, "w1x": w1x, "w2a": w2a_np, "w2b": w2b_np,
            "w3c": w3c, "b12": b12, "ident": ident,
        })

    return nc, in_maps, B, H, Bc


def _assemble(res, B, H, Bc):
    out = np.empty((B, H), np.float32)
    for cid in range(NCORES):
        ob = res.results[cid]["outb"]  # [H, 128, 8]
        oc = ob.reshape(H, 128, 2, 4).transpose(2, 3, 1, 0).reshape(Bc, H)
        out[cid * Bc:(cid + 1) * Bc] = oc
    return out


def kernel(**inputs):
    from concourse.bass_utils import run_bass_kernel_spmd
    nc, in_maps, B, H, Bc = _prepare(**inputs)
    res = run_bass_kernel_spmd(nc, in_maps, core_ids=list(range(NCORES)))
    return _assemble(res, B, H, Bc)

